# revision 1
# baseline (speedup 1.0000x reference)
"""Trainium2 Bass kernel for nn_AttentionHead (B=4, S=4096, H=1024, D=64).

Reference computation (note the unusual K-first ordering):
    K = x @ Wk.T; Q = x @ Wq.T; V = x @ Wv.T            [B,S,D]
    scores[b,i,j] = (K[b,i] . Q[b,j]) / sqrt(D)         [B,S,S]
    scores[:, :, j] = -1e12 where mask[:, j] == 0
    out = softmax(scores, axis=2) @ V                   [B,S,D]

Sharding: 8 cores = 4 batches x 2 key-row chunks of 2048. Each core gets a
batch's x ROLLED so its own key rows are always rows [0, 2048) — the SPMD
program is identical across cores. Softmax runs over the full (rolled) query
axis on every core, so rolling is correctness-neutral.

Per-core pipeline (bf16 matmuls, fp32 accumulation):
  - x streams in fp32 natural layout as 32 [128, 1024] slabs on both DMA
    queues, is cast to bf16 on DVE, and transposed on the PE into SBUF x^T.
    Slab pairs interleave their identity-matmul transposes across two PSUM
    banks (back-to-back same-bank transposes serialize at ~215ns vs ~95ns
    alternating). This reads x from HBM exactly once — much faster than a
    DMA-xbar transpose, which needs an fp32->bf16 DRAM round trip first.
  - One [Wq|Wv] stationary gives Q^T (rows 0:64) and V^T (rows 64:128) per
    512-col block; K^T separately over own 2048 rows; V^T -> V via PE
    transposes. V gets a ones column (softmax denominator).
  - PE warmup matmuls on junk data cover the DMA ramp so the HAM clock-gate
    sits at 8/8 when real work arrives.
  - Two passes over query tiles t=0..31 (one per 1024-wide i-half). Per
    slot: scores^T = Q^T_t.T @ K^T on PE; exp(0.125*s + maskbias[j]) on ACT
    (mask folded into the per-partition bias; masked queries underflow to
    exactly 0); PE accumulates V'_t.T @ P^T_t into out'^T [65, 1024] —
    rows 0:64 numerator^T, row 64 denominator, one PSUM bank per 512-col
    window. The AV matmuls are emitted one slot BEHIND the scores matmuls:
    otherwise they head-of-line block the PE queue waiting on exp. Pass A
    is emission-interleaved with the slab/projection stream so PE always
    has dense work chasing the DMA.
  - Per-pass finale: PE-transpose out'^T via identity matmul, then
    out = numerator * reciprocal(denominator) on DVE; one DMA store.
"""

import numpy as np

B, S, H, D = 4, 4096, 1024, 64
N_CORES = 8
SC = S // 2  # key rows per core
HC = H // 128  # contraction chunks
JT = S // 128  # query tiles
SL = S // 128  # x slabs
NEG = -30000.0
N_WARM = 26

_CACHE = {}


def _build():
    import concourse.bass as bass
    import concourse.tile as tile
    from concourse import bacc, mybir

    dt = mybir.dt
    AF = mybir.ActivationFunctionType

    nc = bacc.Bacc(
        "TRN2", target_bir_lowering=False, debug=False, num_devices=N_CORES
    )
    x = nc.dram_tensor("x", [S, H], dt.float32, kind="ExternalInput").ap()
    wqv = nc.dram_tensor("wqv", [H, 2 * D], dt.float32, kind="ExternalInput").ap()
    wkt = nc.dram_tensor("wkt", [H, D], dt.float32, kind="ExternalInput").ap()
    mb = nc.dram_tensor("mb", [128, JT], dt.float32, kind="ExternalInput").ap()
    ident = nc.dram_tensor("ident", [128, 128], dt.float32, kind="ExternalInput").ap()
    out = nc.dram_tensor("out", [SC, D], dt.float32, kind="ExternalOutput").ap()

    with (
        tile.TileContext(nc) as tc,
        tc.tile_pool(name="persist", bufs=1) as persist,
        tc.tile_pool(name="slabf", bufs=4) as slabf,
        tc.tile_pool(name="slabb", bufs=4) as slabb,
        tc.tile_pool(name="ptile", bufs=6) as ptile,
        tc.tile_pool(name="accs", bufs=2) as accs,
        tc.tile_pool(name="fin", bufs=2) as fin,
    ):
        qt = persist.tile([128, S], dt.bfloat16)  # rows 0:64 = Q^T
        kt = persist.tile([128, SC], dt.bfloat16)  # rows 0:64 = K^T
        vtsb = persist.tile([128, S], dt.bfloat16)  # rows 64:128 = V^T
        vp = persist.tile([128, JT, D + 1], dt.bfloat16)
        mb_sb = persist.tile([128, JT], dt.float32)
        id_f32 = persist.tile([128, 128], dt.float32)
        id_bf = persist.tile([128, 128], dt.bfloat16)
        wtile = persist.tile([128, 512], dt.bfloat16)
        xT = persist.tile([128, HC, S], dt.bfloat16)
        wqv_sb = persist.tile([128, HC, 2 * D], dt.bfloat16)
        wk_sb = persist.tile([128, HC, D], dt.bfloat16)

        nc.vector.memset(vp[:, :, D], 1.0)
        nc.vector.memset(wtile[:], 0.0)
        nc.gpsimd.dma_start(id_bf[:], ident[:])

        def small_loads():
            nc.sync.dma_start(mb_sb[:], mb[:])
            nc.sync.dma_start(id_f32[:], ident[:])
            nc.gpsimd.dma_start(wqv_sb[:], wqv.rearrange("(c p) d -> p c d", p=128))
            nc.gpsimd.dma_start(wk_sb[:], wkt.rearrange("(c p) d -> p c d", p=128))

        with (
            tc.tile_pool(name="psco", bufs=2, space="PSUM") as psco,
            tc.tile_pool(name="ppx", bufs=2, space="PSUM") as ppx,
            tc.tile_pool(name="pacc", bufs=1, space="PSUM") as pacc,
        ):

            # --- x slab pair: load (both queues) + DVE cast + PE transpose,
            # transposes interleaved across two PSUM banks ---
            def slab_pair(sp):
                sls = [2 * sp, 2 * sp + 1]
                xbs, pxts = [], []
                for k, sl in enumerate(sls):
                    xf = slabf.tile([128, H], dt.float32, tag=f"xf{k}")
                    eng = nc.gpsimd if k == 0 else nc.sync
                    eng.dma_start(xf[:], x[128 * sl : 128 * (sl + 1), :])
                    xb = slabb.tile([128, H], dt.bfloat16, tag=f"xb{k}")
                    nc.vector.tensor_copy(xb[:], xf[:])
                    xbs.append(xb)
                    pxt = ppx.tile([128, HC, 128], dt.bfloat16, tag="px")
                    pxts.append(pxt)
                for hc in range(HC):
                    for k in range(2):
                        nc.tensor.matmul(
                            pxts[k][:, hc, :],
                            xbs[k][:, 128 * hc : 128 * (hc + 1)],
                            id_bf[:],
                            is_transpose=True,
                            start=(hc == 0),
                            stop=(hc == HC - 1),
                        )
                for k, sl in enumerate(sls):
                    nc.vector.tensor_copy(
                        xT[:, :, 128 * sl : 128 * (sl + 1)], pxts[k][:]
                    )

            # --- PE work generators ---
            def proj_qv(sb):  # [Q^T; V^T] for one 512-wide s-block
                ps = ppx.tile([128, 512], dt.float32, tag="px")
                for hc in range(HC):
                    nc.tensor.matmul(
                        ps[:],
                        wqv_sb[:, hc, :],
                        xT[:, hc, 512 * sb : 512 * (sb + 1)],
                        start=(hc == 0),
                        stop=(hc == HC - 1),
                    )
                nc.vector.tensor_copy(qt[0:64, 512 * sb : 512 * (sb + 1)], ps[0:64, :])
                nc.vector.tensor_copy(
                    vtsb[64:128, 512 * sb : 512 * (sb + 1)], ps[64:128, :]
                )

            def proj_k(sb):
                ps = ppx.tile([64, 512], dt.float32, tag="px")
                for hc in range(HC):
                    nc.tensor.matmul(
                        ps[:],
                        wk_sb[:, hc, :],
                        xT[:, hc, 512 * sb : 512 * (sb + 1)],
                        start=(hc == 0),
                        stop=(hc == HC - 1),
                    )
                nc.vector.tensor_copy(kt[0:64, 512 * sb : 512 * (sb + 1)], ps[:])

            def vt_block(st0, st1):  # V^T -> V via PE transpose
                for st in range(st0, st1):
                    pvt = ppx.tile([128, D], dt.bfloat16, tag="px")
                    nc.tensor.transpose(
                        pvt[:],
                        vtsb[64:128, 128 * st : 128 * (st + 1)],
                        id_bf[64:128, 64:128],
                    )
                    nc.vector.tensor_copy(vp[:, st, 0:D], pvt[:])

            # --- t-loop slot machinery: AV deferred one slot behind ---
            pending = []

            def flush_av(acc, ih):
                if not pending:
                    return
                pt, t = pending.pop()
                for nb in range(2):
                    nc.tensor.matmul(
                        acc[:, 512 * nb : 512 * (nb + 1)],
                        vp[:, t, :],
                        pt[:, 512 * nb : 512 * (nb + 1)],
                        start=(t == 0),
                        stop=(t == JT - 1),
                    )

            def t_slot(t, acc, ih):
                ps = psco.tile([128, 1024], dt.float32, tag="ps")
                for nb in range(2):
                    nc.tensor.matmul(
                        ps[:, 512 * nb : 512 * (nb + 1)],
                        qt[0:64, 128 * t : 128 * (t + 1)],
                        kt[0:64, 1024 * ih + 512 * nb : 1024 * ih + 512 * (nb + 1)],
                        start=True,
                        stop=True,
                    )
                flush_av(acc, ih)
                pt = ptile.tile([128, 1024], dt.bfloat16)
                nc.scalar.activation(
                    pt[:], ps[:], AF.Exp, bias=mb_sb[:, t : t + 1], scale=0.125
                )
                pending.append((pt, t))

            def finale(acc_sb, ih):
                for k in range(8):
                    po = ppx.tile([128, D + 1], dt.float32, tag="px")
                    nc.tensor.transpose(
                        po[:],
                        acc_sb[:, 128 * k : 128 * (k + 1)],
                        id_f32[0 : D + 1, 0 : D + 1],
                    )
                    rc = fin.tile([128, 1], dt.float32, tag="rc")
                    nc.vector.reciprocal(rc[:], po[:, D : D + 1])
                    nc.vector.tensor_scalar_mul(
                        oall[:, 8 * ih + k, :], po[:, 0:D], rc[:]
                    )

            oall = fin.tile([128, 16, D], dt.float32, tag="oall")

            # ---- pass A (i-half 0) interleaved with slabs + projections ----
            accA = pacc.tile([D + 1, 1024], dt.float32, tag="acc")
            tA = lambda t: t_slot(t, accA, 0)
            slab_pair(0)
            slab_pair(1)
            small_loads()
            # PE warmup while the first slabs stream in
            pw = ppx.tile([128, 512], dt.float32, tag="px")
            for _ in range(N_WARM):
                nc.tensor.matmul(
                    pw[:], wtile[:, 0:128], wtile[:], start=True, stop=True
                )
            dummy = fin.tile([128, 1], dt.float32, tag="dummy")
            nc.scalar.activation(dummy[:], wtile[:, 0:1], AF.Exp)
            proj_qv(0)
            slab_pair(2)
            slab_pair(3)
            proj_qv(1)
            proj_k(0)
            proj_k(1)
            vt_block(0, 4)
            slab_pair(4)
            slab_pair(5)
            proj_k(2)
            proj_qv(2)
            vt_block(4, 8)
            slab_pair(6)
            slab_pair(7)
            proj_k(3)
            proj_qv(3)
            vt_block(8, 12)
            for t in range(0, 4):
                tA(t)
            slab_pair(8)
            slab_pair(9)
            proj_qv(4)
            vt_block(12, 16)
            for t in range(4, 8):
                tA(t)
            slab_pair(10)
            slab_pair(11)
            proj_qv(5)
            vt_block(16, 20)
            for t in range(8, 12):
                tA(t)
            slab_pair(12)
            slab_pair(13)
            proj_qv(6)
            vt_block(20, 24)
            for t in range(12, 16):
                tA(t)
            slab_pair(14)
            slab_pair(15)
            proj_qv(7)
            vt_block(24, 32)
            for t in range(16, 32):
                tA(t)
            flush_av(accA, 0)
            acc_sbA = accs.tile([D + 1, 1024], dt.float32, tag="accs")
            nc.vector.tensor_copy(acc_sbA[:, 0:512], accA[:, 0:512])
            nc.vector.tensor_copy(acc_sbA[:, 512:1024], accA[:, 512:1024])

            # ---- finale A + pass B (i-half 1) ----
            finale(acc_sbA, 0)
            nc.sync.dma_start(
                out[0:1024, :].rearrange("(k p) d -> p k d", p=128),
                oall[:, 0:8, :],
            )
            accB = pacc.tile([D + 1, 1024], dt.float32, tag="acc")
            for t in range(JT):
                t_slot(t, accB, 1)
            flush_av(accB, 1)
            acc_sbB = accs.tile([D + 1, 1024], dt.float32, tag="accs")
            nc.vector.tensor_copy(acc_sbB[:, 0:512], accB[:, 0:512])
            nc.vector.tensor_copy(acc_sbB[:, 512:1024], accB[:, 512:1024])
            finale(acc_sbB, 1)
            nc.sync.dma_start(
                out[1024:2048, :].rearrange("(k p) d -> p k d", p=128),
                oall[:, 8:16, :],
            )

    nc.compile()
    return nc


def _in_maps(x, mask, Wk, Wq, Wv):
    wqv = np.ascontiguousarray(np.concatenate([Wq.T, Wv.T], axis=1), dtype=np.float32)
    wkt = np.ascontiguousarray(Wk.T, dtype=np.float32)
    ident = np.eye(128, dtype=np.float32)
    maps = []
    for c in range(N_CORES):
        b, half = c // 2, c % 2
        i0 = half * SC
        xr = np.ascontiguousarray(np.roll(x[b], -i0, axis=0))
        mr = np.roll(mask[b], -i0)
        mbv = np.where(mr == 0, np.float32(NEG), np.float32(0.0)).astype(np.float32)
        mbt = np.ascontiguousarray(mbv.reshape(JT, 128).T)  # [128, JT], j = 128*t + p
        maps.append({"x": xr, "wqv": wqv, "wkt": wkt, "mb": mbt, "ident": ident})
    return maps


def kernel(x, mask, Wk, Wq, Wv):
    from concourse.bass_utils import run_bass_kernel_spmd

    if "nc" not in _CACHE:
        _CACHE["nc"] = _build()
    nc = _CACHE["nc"]
    maps = _in_maps(x, mask, Wk, Wq, Wv)
    br = run_bass_kernel_spmd(nc, maps, list(range(N_CORES)))
    out = np.empty((B, S, D), dtype=np.float32)
    for c in range(N_CORES):
        b, half = c // 2, c % 2
        out[b, half * SC : (half + 1) * SC, :] = br.results[c]["out"]
    return out



# revision 4
# speedup vs baseline: 1.7120x; 1.7120x over previous
"""Trainium2 Bass kernel for nn_AttentionHead (B=4, S=4096, H=1024, D=64).

Reference computation (note the unusual K-first ordering):
    K = x @ Wk.T; Q = x @ Wq.T; V = x @ Wv.T            [B,S,D]
    scores[b,i,j] = (K[b,i] . Q[b,j]) / sqrt(D)         [B,S,S]
    scores[:, :, j] = -1e12 where mask[:, j] == 0
    out = softmax(scores, axis=2) @ V                   [B,S,S] @ [B,S,D]

Key observations exploited here:
  - Masked j-columns get softmax weight EXACTLY 0 (exp underflows), so the
    host drops them up front: the query/value axis is compacted from the
    mask (~2048 of 4096 survive) and padded to a fixed J (2304 by default;
    the build is parameterized on J as a fallback for denser masks). This
    halves the scores/exp/AV work, which dominates.
  - x^T in bf16 is pure data movement, so the host ships it pre-transposed
    (like the baseline's host-side roll): no on-chip transposes of x, no
    fp32->bf16 casts, and half the HBM traffic.

Sharding: 8 cores = 4 batches x 2 key-row halves of 2048. Each core gets
x^T for its own 2048 key rows (xtk) plus the batch-shared mask-compacted
x^T for queries/values (xtq).

Per-core pipeline (bf16 matmuls, fp32 accumulation):
  - One [Wq|Wv] stationary gives Q^T (rows 0:64) and V^T (rows 64:128) per
    query-column block; K^T over own 2048 rows; V^T -> V via PE
    transposes. V gets a ones column (softmax denominator).
  - PE warmup matmuls on junk data cover the DMA ramp so the HAM
    clock-gate sits at 8/8 when real work arrives.
  - Two passes over query tiles t=0..JT-1 (one per 1024-wide i-half). Per
    slot: scores^T = Q^T_t.T @ K^T on PE; exp(0.125*s + maskbias[j]) on
    ACT (mask/pad folded into the per-partition bias; masked queries
    underflow to exactly 0); PE accumulates V'_t.T @ P^T_t into out'^T
    [65, 1024] - rows 0:64 numerator^T, row 64 denominator, one PSUM bank
    pair per pass. The AV matmuls are emitted one slot BEHIND the scores
    matmuls: otherwise they head-of-line block the PE queue waiting on
    exp. Pass A is emission-interleaved with the projection stream so PE
    always has dense work chasing the DMA.
  - Per-pass finale: PE-transpose out'^T via identity matmul, then
    out = numerator * reciprocal(denominator) on DVE; one DMA store.
"""

import numpy as np

B, S, H, D = 4, 4096, 1024, 64
N_CORES = 8
SC = S // 2  # key rows (output rows) per core
HC = H // 128  # contraction chunks
J_DEF = 2304  # padded, mask-compacted query-column count
NEG = -30000.0
N_WARM = 26

_CACHE = {}


def _build(J):
    import concourse.tile as tile
    from concourse import bacc, mybir

    dt = mybir.dt
    AF = mybir.ActivationFunctionType
    JT = J // 128
    qblocks = [(c0, min(c0 + 512, J)) for c0 in range(0, J, 512)]

    nc = bacc.Bacc(
        "TRN2", target_bir_lowering=False, debug=False, num_devices=N_CORES
    )
    xtk = nc.dram_tensor("xtk", [H, SC], dt.bfloat16, kind="ExternalInput").ap()
    xtq = nc.dram_tensor("xtq", [H, J], dt.bfloat16, kind="ExternalInput").ap()
    wqv = nc.dram_tensor("wqv", [H, 2 * D], dt.float32, kind="ExternalInput").ap()
    wkt = nc.dram_tensor("wkt", [H, D], dt.float32, kind="ExternalInput").ap()
    mb = nc.dram_tensor("mb", [128, JT], dt.float32, kind="ExternalInput").ap()
    ident = nc.dram_tensor("ident", [128, 128], dt.float32, kind="ExternalInput").ap()
    out = nc.dram_tensor("out", [SC, D], dt.float32, kind="ExternalOutput").ap()

    xtk_r = xtk.rearrange("(c p) s -> p c s", p=128)
    xtq_r = xtq.rearrange("(c p) s -> p c s", p=128)

    with (
        tile.TileContext(nc) as tc,
        tc.tile_pool(name="persist", bufs=1) as persist,
        tc.tile_pool(name="ptile", bufs=6) as ptile,
        tc.tile_pool(name="accs", bufs=2) as accs,
        tc.tile_pool(name="fin", bufs=2) as fin,
    ):
        qt = persist.tile([128, J], dt.bfloat16)  # rows 0:64 = Q^T
        kt = persist.tile([128, SC], dt.bfloat16)  # rows 0:64 = K^T
        vtsb = persist.tile([128, J], dt.bfloat16)  # rows 64:128 = V^T
        vp = persist.tile([128, JT, D + 1], dt.bfloat16)
        mb_sb = persist.tile([128, JT], dt.float32)
        id_f32 = persist.tile([128, 128], dt.float32)
        id_bf = persist.tile([128, 128], dt.bfloat16)
        wtile = persist.tile([128, 512], dt.bfloat16)
        xk_sb = persist.tile([128, HC, SC], dt.bfloat16)
        xq_sb = persist.tile([128, HC, J], dt.bfloat16)
        wqv_sb = persist.tile([128, HC, 2 * D], dt.bfloat16)
        wk_sb = persist.tile([128, HC, D], dt.bfloat16)

        nc.vector.memset(vp[:, :, D], 1.0)
        nc.vector.memset(wtile[:], 0.0)

        with (
            tc.tile_pool(name="psco", bufs=2, space="PSUM") as psco,
            tc.tile_pool(name="ppx", bufs=2, space="PSUM") as ppx,
            tc.tile_pool(name="pacc", bufs=1, space="PSUM") as pacc,
        ):
            # --- DMA queue plans; proj work chases the slices ---
            def big_loads():
                # gpsimd queue: weights for K proj, then alternating slices
                nc.gpsimd.dma_start(id_bf[:], ident[:])
                nc.gpsimd.dma_start(
                    wk_sb[:], wkt.rearrange("(c p) d -> p c d", p=128)
                )
                nc.gpsimd.dma_start(xk_sb[:, :, 0:512], xtk_r[:, :, 0:512])
                nc.gpsimd.dma_start(
                    wqv_sb[:], wqv.rearrange("(c p) d -> p c d", p=128)
                )
                nc.gpsimd.dma_start(xk_sb[:, :, 1024:1536], xtk_r[:, :, 1024:1536])
                c0, c1 = qblocks[0]
                nc.gpsimd.dma_start(xq_sb[:, :, c0:c1], xtq_r[:, :, c0:c1])
                for c0, c1 in qblocks[2::2]:
                    nc.gpsimd.dma_start(xq_sb[:, :, c0:c1], xtq_r[:, :, c0:c1])
                # sync queue
                nc.sync.dma_start(mb_sb[:], mb[:])
                nc.sync.dma_start(id_f32[:], ident[:])
                nc.sync.dma_start(xk_sb[:, :, 512:1024], xtk_r[:, :, 512:1024])
                c0, c1 = qblocks[1]
                nc.sync.dma_start(xq_sb[:, :, c0:c1], xtq_r[:, :, c0:c1])
                nc.sync.dma_start(xk_sb[:, :, 1536:2048], xtk_r[:, :, 1536:2048])
                for c0, c1 in qblocks[3::2]:
                    nc.sync.dma_start(xq_sb[:, :, c0:c1], xtq_r[:, :, c0:c1])

            # --- PE work generators ---
            def proj_qv(bi):  # [Q^T; V^T] for one query-column block
                c0, c1 = qblocks[bi]
                ps = ppx.tile([128, c1 - c0], dt.float32, tag="px")
                for hc in range(HC):
                    nc.tensor.matmul(
                        ps[:],
                        wqv_sb[:, hc, :],
                        xq_sb[:, hc, c0:c1],
                        start=(hc == 0),
                        stop=(hc == HC - 1),
                    )
                nc.vector.tensor_copy(qt[0:64, c0:c1], ps[0:64, :])
                nc.vector.tensor_copy(vtsb[64:128, c0:c1], ps[64:128, :])

            def proj_k(sb):
                ps = ppx.tile([64, 512], dt.float32, tag="px")
                for hc in range(HC):
                    nc.tensor.matmul(
                        ps[:],
                        wk_sb[:, hc, :],
                        xk_sb[:, hc, 512 * sb : 512 * (sb + 1)],
                        start=(hc == 0),
                        stop=(hc == HC - 1),
                    )
                nc.vector.tensor_copy(kt[0:64, 512 * sb : 512 * (sb + 1)], ps[:])

            def vt_block(st0, st1):  # V^T -> V via PE transpose
                for st in range(st0, st1):
                    pvt = ppx.tile([128, D], dt.bfloat16, tag="px")
                    nc.tensor.transpose(
                        pvt[:],
                        vtsb[64:128, 128 * st : 128 * (st + 1)],
                        id_bf[64:128, 64:128],
                    )
                    nc.vector.tensor_copy(vp[:, st, 0:D], pvt[:])

            # --- t-loop slot machinery: AV deferred one slot behind ---
            pending = []

            def flush_av(acc):
                if not pending:
                    return
                pt, t = pending.pop()
                for nb in range(2):
                    nc.tensor.matmul(
                        acc[:, 512 * nb : 512 * (nb + 1)],
                        vp[:, t, :],
                        pt[:, 512 * nb : 512 * (nb + 1)],
                        start=(t == 0),
                        stop=(t == JT - 1),
                    )

            def t_slot(t, acc, ih):
                ps = psco.tile([128, 1024], dt.float32, tag="ps")
                for nb in range(2):
                    nc.tensor.matmul(
                        ps[:, 512 * nb : 512 * (nb + 1)],
                        qt[0:64, 128 * t : 128 * (t + 1)],
                        kt[0:64, 1024 * ih + 512 * nb : 1024 * ih + 512 * (nb + 1)],
                        start=True,
                        stop=True,
                    )
                flush_av(acc)
                pt = ptile.tile([128, 1024], dt.bfloat16)
                nc.scalar.activation(
                    pt[:], ps[:], AF.Exp, bias=mb_sb[:, t : t + 1], scale=0.125
                )
                pending.append((pt, t))

            def finale(acc_sb, ih):
                for k in range(8):
                    po = ppx.tile([128, D + 1], dt.float32, tag="px")
                    nc.tensor.transpose(
                        po[:],
                        acc_sb[:, 128 * k : 128 * (k + 1)],
                        id_f32[0 : D + 1, 0 : D + 1],
                    )
                    rc = fin.tile([128, 1], dt.float32, tag="rc")
                    nc.vector.reciprocal(rc[:], po[:, D : D + 1])
                    nc.vector.tensor_scalar_mul(
                        oall[:, 8 * ih + k, :], po[:, 0:D], rc[:]
                    )

            oall = fin.tile([128, 16, D], dt.float32, tag="oall")

            # ---- pass A (i-half 0) interleaved with the projections ----
            big_loads()
            accA = pacc.tile([D + 1, 1024], dt.float32, tag="acc")
            tA = lambda t: t_slot(t, accA, 0)
            # PE warmup while the first slices stream in
            pw = ppx.tile([128, 512], dt.float32, tag="px")
            for _ in range(N_WARM):
                nc.tensor.matmul(
                    pw[:], wtile[:, 0:128], wtile[:], start=True, stop=True
                )
            dummy = fin.tile([128, 1], dt.float32, tag="dummy")
            nc.scalar.activation(dummy[:], wtile[:, 0:1], AF.Exp)
            proj_k(0)
            proj_k(1)
            proj_qv(0)
            vt_block(0, 4)
            tA(0)
            tA(1)
            proj_qv(1)
            vt_block(4, 8)
            tA(2)
            tA(3)
            proj_k(2)
            tA(4)
            tA(5)
            proj_k(3)
            tA(6)
            tA(7)
            proj_qv(2)
            vt_block(8, 12)
            for t in range(8, 12):
                tA(t)
            proj_qv(3)
            vt_block(12, 16)
            for t in range(12, 16):
                tA(t)
            if len(qblocks) > 4:
                proj_qv(4)
            vt_block(16, JT)
            for t in range(16, JT):
                tA(t)
            flush_av(accA)
            acc_sbA = accs.tile([D + 1, 1024], dt.float32, tag="accs")
            nc.vector.tensor_copy(acc_sbA[:, 0:512], accA[:, 0:512])
            nc.vector.tensor_copy(acc_sbA[:, 512:1024], accA[:, 512:1024])

            # ---- finale A + pass B (i-half 1) ----
            finale(acc_sbA, 0)
            nc.sync.dma_start(
                out[0:1024, :].rearrange("(k p) d -> p k d", p=128),
                oall[:, 0:8, :],
            )
            accB = pacc.tile([D + 1, 1024], dt.float32, tag="acc")
            for t in range(JT):
                t_slot(t, accB, 1)
            flush_av(accB)
            acc_sbB = accs.tile([D + 1, 1024], dt.float32, tag="accs")
            nc.vector.tensor_copy(acc_sbB[:, 0:512], accB[:, 0:512])
            nc.vector.tensor_copy(acc_sbB[:, 512:1024], accB[:, 512:1024])
            finale(acc_sbB, 1)
            nc.sync.dma_start(
                out[1024:2048, :].rearrange("(k p) d -> p k d", p=128),
                oall[:, 8:16, :],
            )

    nc.compile()
    return nc


def _in_maps(x, mask, Wk, Wq, Wv):
    import ml_dtypes

    bf16 = ml_dtypes.bfloat16
    wqv = np.ascontiguousarray(np.concatenate([Wq.T, Wv.T], axis=1), dtype=np.float32)
    wkt = np.ascontiguousarray(Wk.T, dtype=np.float32)
    ident = np.eye(128, dtype=np.float32)
    nk = [int((mask[b] != 0).sum()) for b in range(B)]
    J = max(J_DEF, -(-max(nk) // 128) * 128)
    JT = J // 128
    xtq_b, mb_b = [], []
    for b in range(B):
        idx = np.flatnonzero(mask[b] != 0)
        xt = np.zeros((H, J), dtype=bf16)
        xt[:, : len(idx)] = x[b].T[:, idx].astype(bf16)
        xtq_b.append(xt)
        mbv = np.full(J, np.float32(NEG), dtype=np.float32)
        mbv[: len(idx)] = 0.0
        mb_b.append(np.ascontiguousarray(mbv.reshape(JT, 128).T))
    maps = []
    for c in range(N_CORES):
        b, half = c // 2, c % 2
        xtk = np.ascontiguousarray(x[b, half * SC : (half + 1) * SC].T.astype(bf16))
        maps.append(
            {
                "xtk": xtk,
                "xtq": xtq_b[b],
                "wqv": wqv,
                "wkt": wkt,
                "mb": mb_b[b],
                "ident": ident,
            }
        )
    return maps, J


def kernel(x, mask, Wk, Wq, Wv):
    from concourse.bass_utils import run_bass_kernel_spmd

    maps, J = _in_maps(x, mask, Wk, Wq, Wv)
    if J not in _CACHE:
        _CACHE[J] = _build(J)
    nc = _CACHE[J]
    br = run_bass_kernel_spmd(nc, maps, list(range(N_CORES)))
    out = np.empty((B, S, D), dtype=np.float32)
    for c in range(N_CORES):
        b, half = c // 2, c % 2
        out[b, half * SC : (half + 1) * SC, :] = br.results[c]["out"]
    return out


# revision 9
# speedup vs baseline: 1.7895x; 1.0453x over previous
"""Trainium2 Bass kernel for nn_AttentionHead (B=4, S=4096, H=1024, D=64).

Reference computation (note the unusual K-first ordering):
    K = x @ Wk.T; Q = x @ Wq.T; V = x @ Wv.T            [B,S,D]
    scores[b,i,j] = (K[b,i] . Q[b,j]) / sqrt(D)         [B,S,S]
    scores[:, :, j] = -1e12 where mask[:, j] == 0
    out = softmax(scores, axis=2) @ V                   [B,S,S] @ [B,S,D]

Key structural choices:
  - Masked j-columns get softmax weight EXACTLY 0 (exp underflows), so the
    host drops them up front: the query/value axis is compacted from the
    mask (~2048 of 4096 survive) and padded to a fixed J (2304 by default;
    the build is parameterized on J as a fallback for denser masks). This
    halves the scores/exp/AV work, which dominates.
  - x^T in bf16 is pure data movement, so the host ships it pre-transposed
    (like the baseline's host-side roll): no on-chip transposes of x, no
    fp32->bf16 casts, and half the HBM traffic. Weights/identities are
    host-cast too, so no DMA needs the (slow, gpsimd-only) cast path and
    bulk input streams across all three DMA trigger queues (gpsimd SWDGE +
    sync/scalar HWDGE).
  - Scores use PE row tiling: contraction is only D=64, so the two 512-wide
    score matmuls of a slot run CONCURRENTLY on row groups 0/1 of the PE
    array (~2x on the scores leg, and their LDWEIGHTS overlap in-flight
    matmuls of the other group). This requires Q^T and K^T duplicated into
    partitions 64:128: K^T comes for free from a [Wk|Wk] stationary
    projection; Q^T is duplicated with one small SBUF->SBUF DMA per block.

Sharding: 8 cores = 4 batches x 2 key-row halves of 2048. Each core gets
x^T for its own 2048 key rows (xtk) plus the batch-shared mask-compacted
x^T for queries/values (xtq).

Per-core pipeline (bf16 matmuls, fp32 accumulation):
  - One [Wq|Wv] stationary gives Q^T (rows 0:64) and V^T (rows 64:128) per
    query-column block; [Wk|Wk] gives duplicated K^T over own 2048 rows;
    V^T -> V via PE transposes. V gets a ones column (softmax denominator).
  - PE warmup matmuls on junk data cover the DMA ramp so the HAM
    clock-gate sits at 8/8 when real work arrives.
  - Two passes over query tiles t=0..JT-1 (one per 1024-wide i-half). Per
    slot: scores^T = Q^T_t.T @ K^T on PE (row-tiled pair); exp(0.125*s +
    maskbias[j]) on ACT (mask/pad folded into the per-partition bias;
    masked queries underflow to exactly 0); PE accumulates V'_t.T @ P^T_t
    into out'^T [65, 1024] - rows 0:64 numerator^T, row 64 denominator.
    The AV matmuls are emitted one slot BEHIND the scores matmuls:
    otherwise they head-of-line block the PE queue waiting on exp. Pass A
    is emission-interleaved with the projection stream so PE always has
    dense work chasing the DMA; pass-B-only data (xtk cols 1024:2048) is
    deprioritized in the DMA queues.
  - Per-pass finale in 512-col chunks: copy acc to SBUF, PE-transpose via
    identity matmul, out = numerator * reciprocal(denominator) on DVE,
    store each chunk immediately.
"""

import numpy as np

B, S, H, D = 4, 4096, 1024, 64
N_CORES = 8
SC = S // 2  # key rows (output rows) per core
HC = H // 128  # contraction chunks
J_MIN = 1024  # floor for the padded, mask-compacted query-column count
NEG = -30000.0
N_WARM = 26

_CACHE = {}


def _build(J):
    import concourse.tile as tile
    from concourse import bacc, mybir

    dt = mybir.dt
    AF = mybir.ActivationFunctionType
    JT = J // 128
    qblocks = [(c0, min(c0 + 512, J)) for c0 in range(0, J, 512)]

    nc = bacc.Bacc(
        "TRN2", target_bir_lowering=False, debug=False, num_devices=N_CORES
    )
    xtk = nc.dram_tensor("xtk", [H, SC], dt.bfloat16, kind="ExternalInput").ap()
    xtq = nc.dram_tensor("xtq", [H, J], dt.bfloat16, kind="ExternalInput").ap()
    wqv = nc.dram_tensor("wqv", [H, 2 * D], dt.bfloat16, kind="ExternalInput").ap()
    wkk = nc.dram_tensor("wkk", [H, 2 * D], dt.bfloat16, kind="ExternalInput").ap()
    mb = nc.dram_tensor("mb", [128, JT], dt.float32, kind="ExternalInput").ap()
    idb = nc.dram_tensor("idb", [128, 128], dt.bfloat16, kind="ExternalInput").ap()
    idf = nc.dram_tensor("idf", [D + 1, D + 1], dt.float32, kind="ExternalInput").ap()
    out = nc.dram_tensor("out", [SC, D], dt.float32, kind="ExternalOutput").ap()

    xtk_r = xtk.rearrange("(c p) s -> p c s", p=128)
    xtq_r = xtq.rearrange("(c p) s -> p c s", p=128)

    with (
        tile.TileContext(nc) as tc,
        tc.tile_pool(name="persist", bufs=1) as persist,
        tc.tile_pool(name="ptile", bufs=6) as ptile,
        tc.tile_pool(name="accs", bufs=2) as accs,
        tc.tile_pool(name="fin", bufs=2) as fin,
    ):
        qt = persist.tile([128, J], dt.bfloat16)  # Q^T duplicated rows 0:64/64:128
        kt = persist.tile([128, SC], dt.bfloat16)  # K^T duplicated rows 0:64/64:128
        vtsb = persist.tile([128, J], dt.bfloat16)  # rows 64:128 = V^T
        vp = persist.tile([128, JT, D + 1], dt.bfloat16)
        mb_sb = persist.tile([128, JT], dt.float32)
        idf_sb = persist.tile([D + 1, D + 1], dt.float32)
        idb_sb = persist.tile([128, 128], dt.bfloat16)
        wtile = persist.tile([128, 512], dt.bfloat16)
        xk_sb = persist.tile([128, HC, SC], dt.bfloat16)
        xq_sb = persist.tile([128, HC, J], dt.bfloat16)
        wqv_sb = persist.tile([128, HC, 2 * D], dt.bfloat16)
        wkk_sb = persist.tile([128, HC, 2 * D], dt.bfloat16)

        nc.vector.memset(vp[:, :, D], 1.0)
        nc.vector.memset(wtile[:], 0.0)

        with (
            tc.tile_pool(name="psco", bufs=2, space="PSUM") as psco,
            tc.tile_pool(name="ppx", bufs=2, space="PSUM") as ppx,
            tc.tile_pool(name="pacc", bufs=1, space="PSUM") as pacc,
        ):
            # --- DMA queue plans; pass-A-critical data first on each queue ---
            def big_loads():
                # pass-A-critical first on each queue; xtk cols 1024:2048
                # (pass-B-only) go last
                nc.gpsimd.dma_start(
                    wkk_sb[:], wkk.rearrange("(c p) d -> p c d", p=128)
                )
                nc.gpsimd.dma_start(xk_sb[:, :, 0:512], xtk_r[:, :, 0:512])
                nc.sync.dma_start(mb_sb[:], mb[:])
                nc.sync.dma_start(idf_sb[:], idf[:])
                nc.sync.dma_start(xk_sb[:, :, 512:1024], xtk_r[:, :, 512:1024])
                nc.scalar.dma_start(idb_sb[:], idb[:])
                nc.scalar.dma_start(
                    wqv_sb[:], wqv.rearrange("(c p) d -> p c d", p=128)
                )
                qv_qs = [nc.scalar, nc.gpsimd, nc.sync]
                for i, (c0, c1) in enumerate(qblocks):
                    qv_qs[i % 3].dma_start(
                        xq_sb[:, :, c0:c1], xtq_r[:, :, c0:c1]
                    )
                nc.gpsimd.dma_start(xk_sb[:, :, 1024:1536], xtk_r[:, :, 1024:1536])
                nc.sync.dma_start(xk_sb[:, :, 1536:2048], xtk_r[:, :, 1536:2048])

            # --- PE work generators ---
            def proj_qv(bi):  # [Q^T; V^T] for one query-column block
                c0, c1 = qblocks[bi]
                ps = ppx.tile([128, c1 - c0], dt.float32, tag="px")
                for hc in range(HC):
                    nc.tensor.matmul(
                        ps[:],
                        wqv_sb[:, hc, :],
                        xq_sb[:, hc, c0:c1],
                        start=(hc == 0),
                        stop=(hc == HC - 1),
                    )
                nc.vector.tensor_copy(qt[0:64, c0:c1], ps[0:64, :])
                nc.vector.tensor_copy(vtsb[64:128, c0:c1], ps[64:128, :])
                # duplicate Q^T into partitions 64:128 for row-tiled scores
                nc.scalar.dma_start(qt[64:128, c0:c1], qt[0:64, c0:c1])

            def proj_k(sb):  # [Wk|Wk] stationary -> K^T in both halves
                ps = ppx.tile([128, 512], dt.float32, tag="px")
                for hc in range(HC):
                    nc.tensor.matmul(
                        ps[:],
                        wkk_sb[:, hc, :],
                        xk_sb[:, hc, 512 * sb : 512 * (sb + 1)],
                        start=(hc == 0),
                        stop=(hc == HC - 1),
                    )
                nc.vector.tensor_copy(kt[:, 512 * sb : 512 * (sb + 1)], ps[:])

            def vt_block(st0, st1):  # V^T -> V via PE transpose
                for st in range(st0, st1):
                    pvt = ppx.tile([128, D], dt.bfloat16, tag="px")
                    nc.tensor.transpose(
                        pvt[:],
                        vtsb[64:128, 128 * st : 128 * (st + 1)],
                        idb_sb[64:128, 64:128],
                    )
                    nc.vector.tensor_copy(vp[:, st, 0:D], pvt[:])

            # --- t-loop slot machinery: AV deferred one slot behind ---
            pending = []

            def flush_av(acc):
                if not pending:
                    return
                pt, t = pending.pop()
                for nb in range(2):
                    nc.tensor.matmul(
                        acc[:, 512 * nb : 512 * (nb + 1)],
                        vp[:, t, :],
                        pt[:, 512 * nb : 512 * (nb + 1)],
                        start=(t == 0),
                        stop=(t == JT - 1),
                    )

            def t_slot(t, acc, ih):
                ps = psco.tile([128, 1024], dt.float32, tag="ps")
                # row-tiled pair: groups 0/1 run concurrently (contraction 64)
                nc.tensor.matmul(
                    ps[:, 0:512],
                    qt[0:64, 128 * t : 128 * (t + 1)],
                    kt[0:64, 1024 * ih : 1024 * ih + 512],
                    start=True,
                    stop=True,
                )
                nc.tensor.matmul(
                    ps[:, 512:1024],
                    qt[64:128, 128 * t : 128 * (t + 1)],
                    kt[64:128, 1024 * ih + 512 : 1024 * ih + 1024],
                    start=True,
                    stop=True,
                )
                flush_av(acc)
                pt = ptile.tile([128, 1024], dt.bfloat16)
                nc.scalar.activation(
                    pt[:], ps[:], AF.Exp, bias=mb_sb[:, t : t + 1], scale=0.125
                )
                pending.append((pt, t))

            def finale(acc, ih):  # chunked: copy, transpose, divide, store
                for h in range(2):
                    acc_sb = accs.tile([D + 1, 512], dt.float32, tag="accs")
                    nc.vector.tensor_copy(acc_sb[:], acc[:, 512 * h : 512 * (h + 1)])
                    for k in range(4):
                        po = ppx.tile([128, D + 1], dt.float32, tag="px")
                        nc.tensor.transpose(
                            po[:],
                            acc_sb[:, 128 * k : 128 * (k + 1)],
                            idf_sb[:],
                        )
                        rc = fin.tile([128, 1], dt.float32, tag="rc")
                        nc.vector.reciprocal(rc[:], po[:, D : D + 1])
                        nc.vector.tensor_scalar_mul(
                            oall[:, 4 * h + k, :], po[:, 0:D], rc[:]
                        )
                    r0 = 1024 * ih + 512 * h
                    nc.sync.dma_start(
                        out[r0 : r0 + 512, :].rearrange("(k p) d -> p k d", p=128),
                        oall[:, 4 * h : 4 * h + 4, :],
                    )

            oall = fin.tile([128, 8, D], dt.float32, tag="oall")

            # ---- pass A (i-half 0) interleaved with the projections ----
            big_loads()
            accA = pacc.tile([D + 1, 1024], dt.float32, tag="acc")
            tA = lambda t: t_slot(t, accA, 0)
            # PE warmup while the first slices stream in
            pw = ppx.tile([128, 512], dt.float32, tag="px")
            for _ in range(N_WARM):
                nc.tensor.matmul(
                    pw[:], wtile[:, 0:128], wtile[:], start=True, stop=True
                )
            dummy = fin.tile([128, 1], dt.float32, tag="dummy")
            nc.scalar.activation(dummy[:], wtile[:, 0:1], AF.Exp)
            proj_k(0)
            proj_k(1)
            proj_qv(0)
            vt_cover = qblocks[0][1] // 128
            vt_block(0, vt_cover)
            next_t = 0
            units = []
            for i in range(1, len(qblocks)):
                units.append(("qv", i))
                if i == 2:
                    units += [("k", 2), ("k", 3)]
            if len(qblocks) <= 2:
                units += [("k", 2), ("k", 3)]
            for kind, i in units:
                if kind == "qv":
                    proj_qv(i)
                    new_cover = qblocks[i][1] // 128
                    vt_block(vt_cover, new_cover)
                    vt_cover = new_cover
                else:
                    proj_k(i)
                tgt = min(vt_cover, next_t + 2)
                while next_t < tgt:
                    tA(next_t)
                    next_t += 1
            while next_t < JT:
                tA(next_t)
                next_t += 1
            flush_av(accA)

            # ---- finale A + pass B (i-half 1) ----
            finale(accA, 0)
            accB = pacc.tile([D + 1, 1024], dt.float32, tag="acc")
            for t in range(JT):
                t_slot(t, accB, 1)
            flush_av(accB)
            finale(accB, 1)

    nc.compile()
    return nc


def _in_maps(x, mask, Wk, Wq, Wv):
    import ml_dtypes

    bf16 = ml_dtypes.bfloat16
    wqv = np.ascontiguousarray(
        np.concatenate([Wq.T, Wv.T], axis=1).astype(bf16)
    )
    wkk = np.ascontiguousarray(np.concatenate([Wk.T, Wk.T], axis=1).astype(bf16))
    idb = np.eye(128, dtype=bf16)
    idf = np.eye(D + 1, dtype=np.float32)
    nk = [int((mask[b] != 0).sum()) for b in range(B)]
    J = max(J_MIN, -(-max(nk) // 128) * 128)
    JT = J // 128
    xtq_b, mb_b = [], []
    for b in range(B):
        idx = np.flatnonzero(mask[b] != 0)
        xt = np.zeros((H, J), dtype=bf16)
        xt[:, : len(idx)] = x[b].T[:, idx].astype(bf16)
        xtq_b.append(xt)
        mbv = np.full(J, np.float32(NEG), dtype=np.float32)
        mbv[: len(idx)] = 0.0
        mb_b.append(np.ascontiguousarray(mbv.reshape(JT, 128).T))
    maps = []
    for c in range(N_CORES):
        b, half = c // 2, c % 2
        xtk = np.ascontiguousarray(x[b, half * SC : (half + 1) * SC].T.astype(bf16))
        maps.append(
            {
                "xtk": xtk,
                "xtq": xtq_b[b],
                "wqv": wqv,
                "wkk": wkk,
                "mb": mb_b[b],
                "idb": idb,
                "idf": idf,
            }
        )
    return maps, J


def kernel(x, mask, Wk, Wq, Wv):
    from concourse.bass_utils import run_bass_kernel_spmd

    maps, J = _in_maps(x, mask, Wk, Wq, Wv)
    if J not in _CACHE:
        _CACHE[J] = _build(J)
    nc = _CACHE[J]
    br = run_bass_kernel_spmd(nc, maps, list(range(N_CORES)))
    out = np.empty((B, S, D), dtype=np.float32)
    for c in range(N_CORES):
        b, half = c // 2, c % 2
        out[b, half * SC : (half + 1) * SC, :] = br.results[c]["out"]
    return out
